# revision 11
# baseline (speedup 1.0000x reference)
"""Branched attention processor (SDXL-like) on 8 Trainium2 NeuronCores.

Sharding: 2-way data-parallel over the half-batch dim x 4-way tensor-parallel
over heads (5 heads = 320 features per core). Each core computes a partial
out^T = Wo[:, c_slice] @ merged[c_slice, :] for its head group; the host sums
the 4 partials per half-batch (the all-reduce) and adds the bias.

Per-core pipeline:
  1. q^T/k^T projections in [o, s] layout (W^T stationary, X^T moving),
     copied back on the Scalar engine as bf16. k^T is stored per head in
     zero-padded [128, 2048] tiles so the QK^T matmuls run with a full
     128-row contraction (64-row PE tile mode runs at half rate on HW).
  2. v projection in natural [s, o] layout; face-branch v gated by mask
     via one strided per-partition-scaled copy per sk tile; a ones column
     per head makes AV also produce the softmax sums.
  3. Attention in the logits^T [sk, sq] layout, 10 uniform rounds of
     (head, sq-half) with the bg/face branch chains interleaved: QK^T
     (bf16, 128-contraction) -> exp on ScalarE with gate/8 folded into the
     activation scale -> AV with v_aug stationary. The softmax tail copies
     the AV accumulator out of PSUM immediately (frees banks for the next
     round), then reciprocal + DMA-to-partition-0 + gpsimd broadcast +
     normalize/merge run on SBUF off the critical path.
  4. Wo partial projection with zero-padded third c-chunk (full-rate),
     PSUM->SBUF copyback on ScalarE, one [128, 2048] store per o-chunk.

Host marshalling: hidden_states pre-transposed to [c, s]; weights pre-sliced
and pre-transposed per core. All 8 cores run one SPMD NEFF.
"""

import numpy as np

import concourse.bass as bass
import concourse.tile as tile
import concourse.mybir as mybir
from concourse import bacc
from concourse.bass_utils import run_bass_kernel_spmd

# Problem shapes (hardcoded per contract)
B2, S, C = 4, 1024, 1280
B = B2 // 2           # 2 half-batches
H = 20                # heads
D = C // H            # 64
G = 4                 # head groups (tensor-parallel)
HG = H // G           # 5 heads per core
OS = HG * D           # 320 features per core
SQ = 2 * S            # 2048 queries per half-batch
P = 128
NCHUNK = C // P       # 10 c-chunks of 128
SKT = S // P          # 8 key tiles of 128

F32 = mybir.dt.float32
F32R = mybir.dt.float32r
BF16 = mybir.dt.bfloat16
EXP = mybir.ActivationFunctionType.Exp


def build_nc(iters: int = 1):
    """Build + compile the SPMD Bass module. iters>1 wraps the body in a
    hardware For_i loop (for timing via the loop-delta method)."""
    nc = bacc.Bacc("TRN2", target_bir_lowering=False, debug=False, num_devices=8)

    # DRAM inputs (per core). xt columns: [q 0:2048 | noise 2048:3072 | ref 3072:4096]
    xt = nc.dram_tensor("xt", [C, 2 * S + 2 * S], F32R, kind="ExternalInput")
    wqT = nc.dram_tensor("wqT", [C, OS], F32R, kind="ExternalInput")
    wkT = nc.dram_tensor("wkT", [C, OS], F32R, kind="ExternalInput")
    wvT = nc.dram_tensor("wvT", [C, OS], F32R, kind="ExternalInput")
    woT = nc.dram_tensor("woT", [OS, C], F32R, kind="ExternalInput")
    g8 = nc.dram_tensor("g8", [P, SKT], F32, kind="ExternalInput")    # gate/8 [p, sk_tile]
    gv = nc.dram_tensor("gv", [P, SKT], F32, kind="ExternalInput")    # raw gate
    outp = nc.dram_tensor("outp", [C, SQ], F32, kind="ExternalOutput")
    import os as _os
    # timing-only ablations (break numerics!)
    abl_skip_attn = bool(_os.environ.get("KSKIP_ATTN"))
    abl_skip_wo = bool(_os.environ.get("KSKIP_WO"))
    abl_no_tail = bool(_os.environ.get("KATT_NOTAIL"))

    with tile.TileContext(nc) as tc:
        with (
            tc.tile_pool(name="persist", bufs=1) as persist,
            tc.tile_pool(name="work", bufs=2) as work,
            tc.tile_pool(name="pt", bufs=2) as ptp,
            tc.tile_pool(name="outsb", bufs=2) as outsb,
            tc.tile_pool(name="ps_l", bufs=2, space="PSUM") as ps_l,
            tc.tile_pool(name="ps_av", bufs=2, space="PSUM") as ps_av,
        ):
            # ---- persistent tiles ----
            # q^T per unit (pairs of heads; unit 2 rows 64:128 zeroed)
            qT = [persist.tile([P, SQ], BF16, tag=f"qT{i}", name=f"qT{i}")
                  for i in range(3)]
            # k^T per head, zero-padded to 128 rows: head h's data lives at
            # rows (h%2)*64..(h%2)*64+64, the other 64 rows stay zero so the
            # QK^T contraction is a full 128 partitions.
            kTz = [persist.tile([P, SQ], BF16, tag=f"kTz{h}", name=f"kTz{h}")
                   for h in range(HG)]
            # merged^T: per unit [128, 2048]; unit 2 rows 64:128 zeroed
            mT = [persist.tile([P, SQ], F32R, tag=f"mT{i}", name=f"mT{i}")
                  for i in range(3)]
            # v_aug per (branch, sk_tile): [128, 5*65]; per head: 64 v cols + ones col
            vaug = [[persist.tile([P, HG * (D + 1)], F32R, tag=f"va{br}_{t}",
                                  name=f"va{br}_{t}")
                     for t in range(SKT)] for br in range(2)]
            # weights resident
            wq_sb = [persist.tile([P, OS], F32R, tag=f"wq{cc}", name=f"wq{cc}") for cc in range(NCHUNK)]
            wk_sb = [persist.tile([P, OS], F32R, tag=f"wk{cc}", name=f"wk{cc}") for cc in range(NCHUNK)]
            wv_sb = [persist.tile([P, OS], F32R, tag=f"wv{cc}", name=f"wv{cc}") for cc in range(NCHUNK)]
            # woT tiles: rows 0:128, 128:256, 256:320 (third padded with zeros)
            wo_sb = [persist.tile([P, C], F32R, tag=f"wo{i}", name=f"wo{i}")
                     for i in range(3)]
            g8_sb = persist.tile([P, SKT], F32, tag="g8")
            gv_sb = persist.tile([P, SKT], F32, tag="gv")

            # weight/gate loads + zero pads (outside the timing loop)
            for cc in range(NCHUNK):
                nc.sync.dma_start(wq_sb[cc][:], wqT[cc * P:(cc + 1) * P, :])
                nc.sync.dma_start(wk_sb[cc][:], wkT[cc * P:(cc + 1) * P, :])
                nc.sync.dma_start(wv_sb[cc][:], wvT[cc * P:(cc + 1) * P, :])
            for i in range(3):
                r0 = i * 128
                r1 = min(OS, r0 + 128)
                nc.sync.dma_start(wo_sb[i][:r1 - r0, :], woT[r0:r1, :])
            nc.vector.memset(wo_sb[2][64:128, :].bitcast(F32), 0.0)
            nc.sync.dma_start(g8_sb[:], g8[:, :])
            nc.sync.dma_start(gv_sb[:], gv[:, :])
            # zero pads written once: qT[2]/mT[2] rows 64:128, kTz pad rows
            nc.vector.memset(qT[2][64:128, :].bitcast(mybir.dt.uint16), 0)
            nc.vector.memset(mT[2][64:128, :].bitcast(F32), 0.0)
            for h in range(HG):
                z0 = 64 if h % 2 == 0 else 0
                nc.vector.memset(kTz[h][z0:z0 + 64, :].bitcast(mybir.dt.uint16), 0)
            # ones columns of v_aug (set once)
            for br in range(2):
                for t in range(SKT):
                    for h in range(HG):
                        nc.vector.memset(
                            vaug[br][t][:, h * (D + 1) + D:h * (D + 1) + D + 1].bitcast(F32), 1.0)

            def body(_iv=None):
                CW = 256  # xt chunk width
                # ---- projections ----
                # region 0: xt cols 0:2048 -> q^T ; region 1: cols 2048:4096 -> k^T, v
                for region in (1, 0):
                    for sch in range(SQ // CW):
                        col0 = region * SQ + sch * CW
                        xta = work.tile([P, NCHUNK * CW], F32R, tag="xta",
                                        name="xta", bufs=3)
                        nc.sync.dma_start(
                            xta[:].rearrange("p (g c) -> p g c", c=CW),
                            xt[0:C, col0:col0 + CW].rearrange(
                                "(g p) c -> p g c", p=P))
                        w_sb = wq_sb if region == 0 else wk_sb
                        for blk in range(3):
                            m0 = blk * 128
                            m1 = min(OS, m0 + 128)
                            ps = ps_l.tile([P, 1024], F32, tag="L",
                                           name="pqk")[:m1 - m0, :CW]
                            for cc in range(NCHUNK):
                                nc.tensor.matmul(
                                    ps, w_sb[cc][:, m0:m1],
                                    xta[:, cc * CW:(cc + 1) * CW],
                                    start=(cc == 0), stop=(cc == NCHUNK - 1))
                            cols = slice(sch * CW, (sch + 1) * CW)
                            if region == 0:
                                nc.scalar.copy(qT[blk][:m1 - m0, cols], ps)
                            else:
                                h0 = blk * 2
                                nc.scalar.copy(kTz[h0][0:64, cols], ps[0:64, :])
                                if m1 - m0 > 64:
                                    nc.scalar.copy(kTz[h0 + 1][64:128, cols],
                                                   ps[64:128, :])
                        if region == 1:
                            # v projection from the same resident xt chunk
                            br = sch // 4
                            for st in range(CW // P):
                                t_idx = (sch % 4) * 2 + st
                                ps = ps_av.tile([P, 1024], F32, tag="av",
                                                name="pv")[:, :OS]
                                for cc in range(NCHUNK):
                                    nc.tensor.matmul(
                                        ps[:],
                                        xta[:, cc * CW + st * P:
                                            cc * CW + (st + 1) * P],
                                        wv_sb[cc][:],
                                        start=(cc == 0), stop=(cc == NCHUNK - 1))
                                va = vaug[br][t_idx]
                                # one strided copy covering all 5 per-head
                                # 64-col blocks (skipping the ones columns)
                                dst = va[:, 0:HG * (D + 1)].rearrange(
                                    "p (g c) -> p g c", c=D + 1)[:, :, 0:D]
                                src = ps[:, 0:OS].rearrange(
                                    "p (g c) -> p g c", c=D)
                                if br == 0:
                                    nc.vector.tensor_copy(dst, src)
                                else:
                                    nc.vector.tensor_scalar_mul(
                                        dst, src, gv_sb[:, t_idx:t_idx + 1])

                # ---- attention: 10 uniform (head, sq-half) rounds ----
                for sqh in range(2):
                    for h in range((0 if abl_skip_attn else HG)):
                        i = h // 2
                        q0 = sqh * 1024
                        avps = [ps_av.tile([P, 1024], F32, tag="av", name="av")
                                for _ in range(2)]
                        for sk in range(SKT):
                            for br in range(2):
                                kcol = br * S + sk * P
                                L = ps_l.tile([P, 1024], F32, tag="L", name="L")
                                for n2 in range(2):
                                    nc.tensor.matmul(
                                        L[:, n2 * 512:(n2 + 1) * 512],
                                        kTz[h][:, kcol:kcol + P],
                                        qT[i][:, q0 + n2 * 512:
                                              q0 + (n2 + 1) * 512],
                                        start=True, stop=True)
                                pt = ptp.tile([P, 1024], F32R, tag="pt",
                                              name="pt")
                                if br == 0:
                                    nc.scalar.activation(
                                        pt[:], L[:], EXP, scale=0.125)
                                else:
                                    nc.scalar.activation(
                                        pt[:], L[:], EXP,
                                        scale=g8_sb[:, sk:sk + 1])
                                va = vaug[br][sk][:, h * (D + 1):
                                                  h * (D + 1) + D + 1]
                                for n2 in range(2):
                                    nc.tensor.matmul(
                                        avps[br][:D + 1,
                                                 n2 * 512:(n2 + 1) * 512],
                                        va, pt[:, n2 * 512:(n2 + 1) * 512],
                                        start=(sk == 0), stop=(sk == SKT - 1))
                        # ---- softmax tail ----
                        # copy accumulators out of PSUM first (frees banks)
                        avsb = [work.tile([65, 1024], F32R, tag="avsb",
                                          name=f"avsb{br}") for br in range(2)]
                        for br in range(2):
                            nc.vector.tensor_copy(avsb[br][:],
                                                  avps[br][:D + 1, :])
                        if abl_no_tail:
                            nc.sync.dma_start(
                                mT[i][(h % 2) * 64:(h % 2) * 64 + 64,
                                      q0:q0 + 1024],
                                avsb[0][0:64, :])
                            continue
                        rbs = [work.tile([65, 1024], F32, tag="rb",
                                         name=f"rb{br}") for br in range(2)]
                        for br in range(2):
                            nc.vector.reciprocal(rbs[br][64:65, :],
                                                 avsb[br][64:65, :])
                            s0 = work.tile([1, 1024], F32, tag="s0",
                                           name="s0")
                            nc.sync.dma_start(s0[0:1, :], rbs[br][64:65, :])
                            nc.gpsimd.partition_broadcast(rbs[br][0:64, :],
                                                          s0[0:1, :])
                        nc.gpsimd.tensor_tensor(
                            avsb[1][0:64, :], avsb[1][0:64, :],
                            rbs[1][0:64, :], mybir.AluOpType.mult)
                        if h % 2 == 0:
                            dst = mT[i][0:64, q0:q0 + 1024]
                            nc.vector.tensor_tensor(
                                dst, avsb[0][0:64, :], rbs[0][0:64, :],
                                mybir.AluOpType.mult)
                            nc.vector.tensor_add(dst, dst, avsb[1][0:64, :])
                        else:
                            nc.vector.tensor_tensor(
                                avsb[0][0:64, :], avsb[0][0:64, :],
                                rbs[0][0:64, :], mybir.AluOpType.mult)
                            nc.vector.tensor_add(avsb[0][0:64, :],
                                                 avsb[0][0:64, :],
                                                 avsb[1][0:64, :])
                            nc.sync.dma_start(
                                mT[i][64:128, q0:q0 + 1024],
                                avsb[0][0:64, :])

                # ---- Wo partial: outp^T[o, sq] = sum_c woT[c, o] * mT[c, sq] ----
                for ot in range(0 if abl_skip_wo else NCHUNK):
                    o0 = ot * P
                    pss = [ps_av.tile([P, 1024], F32, tag="av", name="avwo")
                           for _ in range(2)]
                    for cc in range(3):
                        for sqh in range(2):
                            for n2 in range(2):
                                c0 = sqh * 1024 + n2 * 512
                                nc.tensor.matmul(
                                    pss[sqh][:, n2 * 512:(n2 + 1) * 512],
                                    wo_sb[cc][:, o0:o0 + P],
                                    mT[cc][:, c0:c0 + 512],
                                    start=(cc == 0), stop=(cc == 2))
                    ob = outsb.tile([P, SQ], F32, tag="ob", name="ob")
                    nc.scalar.copy(ob[:, 0:1024], pss[0][:])
                    nc.vector.tensor_copy(ob[:, 1024:2048], pss[1][:])
                    nc.sync.dma_start(outp[o0:o0 + P, :], ob[:])

            if iters > 1:
                with tc.For_i(0, iters, 1):
                    body()
            else:
                body()

    nc.compile()
    return nc


_NC_CACHE = {}


def _get_nc(iters: int = 1):
    if iters not in _NC_CACHE:
        _NC_CACHE[iters] = build_nc(iters)
    return _NC_CACHE[iters]


def make_in_maps(hidden_states, mask_ref, Wq, Wk, Wv, Wo):
    hsT = np.ascontiguousarray(
        np.asarray(hidden_states, dtype=np.float32).transpose(0, 2, 1))  # [4, C, S]
    mask = np.asarray(mask_ref, dtype=np.float32)
    Wq = np.asarray(Wq, dtype=np.float32)
    Wk = np.asarray(Wk, dtype=np.float32)
    Wv = np.asarray(Wv, dtype=np.float32)
    Wo = np.asarray(Wo, dtype=np.float32)
    in_maps = []
    for b in range(B):
        xt_b = np.concatenate(
            [hsT[2 * b], hsT[2 * b + 1], hsT[b], hsT[2 + b]], axis=1)  # [C, 4096]
        gate = mask[b, :, 0]                        # [S]
        gcol = np.ascontiguousarray(gate.reshape(SKT, P).T)  # [128, 8]
        for g in range(G):
            osl = slice(g * OS, (g + 1) * OS)
            in_maps.append({
                "xt": np.ascontiguousarray(xt_b),
                "wqT": np.ascontiguousarray(Wq[osl, :].T),
                "wkT": np.ascontiguousarray(Wk[osl, :].T),
                "wvT": np.ascontiguousarray(Wv[osl, :].T),
                "woT": np.ascontiguousarray(Wo[:, osl].T),
                "g8": gcol * 0.125,
                "gv": gcol,
            })
    return in_maps


def kernel(hidden_states, mask_ref, Wq, Wk, Wv, Wo, bo, heads):
    assert int(heads) == H
    nc = _get_nc(1)
    in_maps = make_in_maps(hidden_states, mask_ref, Wq, Wk, Wv, Wo)
    res = run_bass_kernel_spmd(nc, in_maps, core_ids=list(range(8)))
    bo = np.asarray(bo, dtype=np.float32)
    out = np.empty((B, SQ, C), dtype=np.float32)
    for b in range(B):
        acc = res.results[b * G]["outp"].copy()
        for g in range(1, G):
            acc += res.results[b * G + g]["outp"]
        out[b] = acc.T + bo
    return out
